# revision 8
# baseline (speedup 1.0000x reference)
"""Trainium2 Bass kernel for top-1 MoE expert MLP (nn_Experts problem).

Strategy (expert-parallel, one expert per NeuronCore):
  - Routing is one-hot top-1: each token is processed by exactly one expert,
    so each core computes the MLP only for the tokens routed to its expert.
  - Host-side shard step: compute token->expert assignment from
    dispatch_tensor, gather each expert's tokens (transposed to [D, CAP]),
    and pack per-core inputs into one contiguous [D, XW] array per expert
    holding xT | w1 | w2 | b1 | gates.  One DMA per 128-row k-block brings
    everything on chip (8 contiguous ~2.4MB DMAs), which also keeps every
    fp32 matmul at <=1 semaphore wait (the fused 4-byte weight-load
    instruction has room for only one).
  - Device: h^T[F,CAP] = gelu(w1^T @ xT + b1); y[CAP,D] = (h @ w2) * gate.
  - Host-side unshard step: scatter per-expert rows back to token order and
    add the shared output bias b2.  No cross-core reduction is needed since
    token outputs are disjoint across experts.
"""

import numpy as np

B, N, D, E, F = 8, 512, 1024, 8, 2048
T = B * N
P = 128
CAP = 640            # per-expert token capacity (max observed ~549 for T=4096, E=8)
NT = CAP // P        # 5 token tiles per expert
KT1 = D // P         # 8  k-tiles for matmul1 (contract over D)
MT1 = F // P         # 16 m-tiles for matmul1 / k-tiles for matmul2
MM_DT = "float32"    # matmul dtype: "float32" (exact) or "float32r" (fast)

# column layout of the packed per-k-block input (all float32)
C_X = 0                        # xT block   [:, 0:CAP]
C_W1 = CAP                     # w1 block   [:, CAP:CAP+F]
C_W2 = CAP + F                 # w2 blocks  [:, C_W2:C_W2+2*D] (2 k2-tiles)
C_B1 = C_W2 + 2 * D            # b1         [:, C_B1:C_B1+MT1] (k-block 0 only)
C_G = C_B1 + MT1               # gates      [:, C_G:C_G+NT]    (k-block 0 only)
XW = -(-(C_G + NT) // 32) * 32  # pad to 128B lines

_NC_CACHE = {}


def _build_bass():
    import concourse.bacc as bacc
    import concourse.tile as tile
    from concourse import mybir

    f32 = mybir.dt.float32
    mm_dt = getattr(mybir.dt, MM_DT)

    def mm(ap):
        return ap.bitcast(mm_dt) if mm_dt is not f32 else ap

    nc = bacc.Bacc(None, target_bir_lowering=False)
    xw = nc.declare_dram_parameter("xw", [D, XW], f32, isOutput=False)
    y = nc.declare_dram_parameter("y", [CAP, D], f32, isOutput=True)

    with tile.TileContext(nc) as tc:
        with (
            tc.tile_pool(name="xwp", bufs=KT1) as xwp,
            tc.tile_pool(name="hp", bufs=MT1) as hp,
            tc.tile_pool(name="stp", bufs=2) as stp,
            tc.tile_pool(name="psA", bufs=2, space="PSUM") as psA,
            tc.tile_pool(name="psB", bufs=4, space="PSUM") as psB,
        ):
            xw_sb = []
            for k in range(KT1):
                t = xwp.tile([P, XW], f32, tag="xw", name=f"xw_{k}")
                nc.sync.dma_start(out=t[:], in_=xw[k * P:(k + 1) * P, :])
                xw_sb.append(t)

            # Phase A: h^T[F, CAP] = gelu(w1^T @ x^T + b1)
            gelu = mybir.ActivationFunctionType.Gelu
            h_sb = []
            for m1 in range(MT1):
                ps0 = psA.tile([P, 512], f32, tag="psA0", name=f"psA0_{m1}")
                ps1 = psA.tile([P, CAP - 512], f32, tag="psA1", name=f"psA1_{m1}")
                for k in range(KT1):
                    lhs = mm(xw_sb[k][:, C_W1 + m1 * P:C_W1 + (m1 + 1) * P])
                    st, sp = (k == 0), (k == KT1 - 1)
                    nc.tensor.matmul(ps0[:], lhs,
                                     mm(xw_sb[k][:, C_X:C_X + 512]),
                                     start=st, stop=sp)
                    nc.tensor.matmul(ps1[:], lhs,
                                     mm(xw_sb[k][:, C_X + 512:C_X + CAP]),
                                     start=st, stop=sp)
                h = hp.tile([P, CAP], f32, tag="h", name=f"h_{m1}")
                bias = xw_sb[0][:, C_B1 + m1:C_B1 + m1 + 1]
                nc.scalar.activation(h[:, 0:512], ps0[:], gelu, bias=bias)
                nc.scalar.activation(h[:, 512:CAP], ps1[:], gelu, bias=bias)
                h_sb.append(h)

            # Phase B: y[CAP, D] = (h @ w2) * gate
            for m in range(NT):
                pss = [psB.tile([P, 512], f32, tag="psB", name=f"psB_{m}_{n}")
                       for n in range(2)]
                for k2 in range(MT1):
                    lhs = mm(h_sb[k2][:, m * P:(m + 1) * P])
                    w2base = C_W2 + (k2 % 2) * D
                    st, sp = (k2 == 0), (k2 == MT1 - 1)
                    for n in range(2):
                        nc.tensor.matmul(
                            pss[n][:], lhs,
                            mm(xw_sb[k2 // 2][:, w2base + n * 512:
                                              w2base + (n + 1) * 512]),
                            start=st, stop=sp)
                gate = xw_sb[0][:, C_G + m:C_G + m + 1]
                for n in range(2):
                    for c in range(2):
                        stage = stp.tile([P, 256], f32, tag="stage",
                                         name=f"stage_{m}_{n}_{c}")
                        nc.vector.tensor_scalar_mul(
                            stage[:], pss[n][:, c * 256:(c + 1) * 256], gate)
                        col = n * 512 + c * 256
                        nc.sync.dma_start(
                            out=y[m * P:(m + 1) * P, col:col + 256],
                            in_=stage[:])
    if not nc.is_finalized():
        nc.finalize()
    return nc


def _get_nc():
    if "nc" not in _NC_CACHE:
        _NC_CACHE["nc"] = _build_bass()
    return _NC_CACHE["nc"]


def kernel(x, dispatch_tensor, combine_tensor, w1, b1, w2, b2, **_):
    from concourse.bass_utils import run_bass_kernel_spmd

    x = np.ascontiguousarray(np.asarray(x, dtype=np.float32)).reshape(T, D)
    dispatch = np.asarray(dispatch_tensor, dtype=np.float32).reshape(T, E)
    combine = np.asarray(combine_tensor, dtype=np.float32).reshape(T, E)
    w1 = np.asarray(w1, dtype=np.float32)
    b1 = np.asarray(b1, dtype=np.float32)
    w2 = np.asarray(w2, dtype=np.float32)
    b2 = np.asarray(b2, dtype=np.float32)

    top = dispatch.argmax(-1)
    gate = combine.sum(-1)
    idxs = [np.nonzero(top == e)[0] for e in range(E)]

    in_maps = []
    for e in range(E):
        idx = idxs[e]
        c = len(idx)
        assert c <= CAP, f"expert {e} got {c} tokens > CAP={CAP}"
        xw = np.zeros((D, XW), np.float32)
        xw[:, C_X:C_X + c] = x[idx].T
        xw[:, C_W1:C_W1 + F] = w1[e]
        for k in range(KT1):
            xw[k * P:(k + 1) * P, C_W2:C_W2 + D] = \
                w2[e][(2 * k) * P:(2 * k + 1) * P, :]
            xw[k * P:(k + 1) * P, C_W2 + D:C_W2 + 2 * D] = \
                w2[e][(2 * k + 1) * P:(2 * k + 2) * P, :]
        xw[:P, C_B1:C_B1 + MT1] = b1[e].reshape(MT1, P).T
        g = np.zeros(CAP, np.float32)
        g[:c] = gate[idx]
        xw[:P, C_G:C_G + NT] = g.reshape(NT, P).T
        in_maps.append({"xw": xw})

    global _LAST_IN_MAPS
    _LAST_IN_MAPS = in_maps
    nc = _get_nc()
    res = run_bass_kernel_spmd(nc, in_maps, list(range(E)))

    y_flat = np.empty((T, D), np.float32)
    for e in range(E):
        y_flat[idxs[e]] = res.results[e]["y"][:len(idxs[e])]
    return (y_flat + b2[None, :]).reshape(B, N, D)


# revision 10
# speedup vs baseline: 2.2899x; 2.2899x over previous
"""Trainium2 Bass kernel for top-1 MoE expert MLP (nn_Experts problem).

Strategy (expert-parallel, one expert per NeuronCore):
  - Routing is one-hot top-1: each token is processed by exactly one expert,
    so each core computes the MLP only for the tokens routed to its expert.
  - Host-side shard step: compute token->expert assignment from
    dispatch_tensor, gather each expert's tokens (transposed to [D, CAP]),
    and pack per-core inputs into one contiguous [D, XW] array per expert
    holding xT | w1 | w2.  One DMA per 128-row k-block brings everything on
    chip (8 contiguous ~2.3MB DMAs).
  - Device: h^T[F,CAP] = gelu(w1^T @ xT + b1); y[CAP,D] = (h @ w2) * gate.
  - Host-side unshard step: scatter per-expert rows back to token order and
    add the shared output bias b2.  No cross-core reduction is needed since
    token outputs are disjoint across experts.

MM_DT selects the tensor-engine dtype: "float32" is exact (4 cycles/row),
"float32r" streams at full rate (1 cycle/row, ~4x faster) but rounds matmul
operands to a 12-bit significand (~1e-4 relative).  For float32r the host
pre-rounds the packed operands (round-to-nearest-even at bit 12) so the BIR
verifier sees fp32r-clean inputs.
"""

import numpy as np

B, N, D, E, F = 8, 512, 1024, 8, 2048
T = B * N
P = 128
CAP = 640            # per-expert token capacity (max observed ~549 for T=4096, E=8)
NT = CAP // P        # 5 token tiles per expert
KT1 = D // P         # 8  k-tiles for matmul1 (contract over D)
MT1 = F // P         # 16 m-tiles for matmul1 / k-tiles for matmul2
MM_DT = "float32r"   # matmul dtype: "float32" (exact) or "float32r" (fast)

# column layout of the packed per-k-block input (matmul operands only)
C_X = 0                        # xT block   [:, 0:CAP]
C_W1 = CAP                     # w1 block   [:, CAP:CAP+F]
C_W2 = CAP + F                 # w2 blocks  [:, C_W2:C_W2+2*D] (2 k2-tiles)
XW = C_W2 + 2 * D              # 4736
# phase-A token chunking (fp32r needs moving dim >= 256 for full rate)
CHUNKS_A = ((0, 320), (320, 640))

_NC_CACHE = {}


def _round_fp32r(a):
    """Round-to-nearest-even at mantissa bit 12 (fp32r has 11 explicit
    mantissa bits: the top 20 bits of an fp32 word)."""
    u = a.view(np.uint32)
    lsb = (u >> 12) & 1
    u = u + 0x7FF + lsb
    u &= np.uint32(0xFFFFF000)
    return u.view(np.float32)


def _build_bass():
    import concourse.bacc as bacc
    import concourse.tile as tile
    from concourse import mybir

    f32 = mybir.dt.float32
    mm_dt = getattr(mybir.dt, MM_DT)

    nc = bacc.Bacc(None, target_bir_lowering=False)
    xw = nc.declare_dram_parameter("xw", [D, XW], mm_dt, isOutput=False)
    gb = nc.declare_dram_parameter("gb", [P, MT1 + NT], f32, isOutput=False)
    y = nc.declare_dram_parameter("y", [CAP, D], f32, isOutput=True)

    with tile.TileContext(nc) as tc:
        with (
            tc.tile_pool(name="xwp", bufs=KT1) as xwp,
            tc.tile_pool(name="gbp", bufs=1) as gbp,
            tc.tile_pool(name="hp", bufs=MT1) as hp,
            tc.tile_pool(name="stp", bufs=2) as stp,
            tc.tile_pool(name="psA", bufs=2, space="PSUM") as psA,
            tc.tile_pool(name="psB", bufs=4, space="PSUM") as psB,
        ):
            gb_sb = gbp.tile([P, MT1 + NT], f32)
            nc.sync.dma_start(out=gb_sb[:], in_=gb[:, :])
            xw_sb = []
            for k in range(KT1):
                t = xwp.tile([P, XW], mm_dt, tag="xw", name=f"xw_{k}")
                nc.sync.dma_start(out=t[:], in_=xw[k * P:(k + 1) * P, :])
                xw_sb.append(t)

            # Phase A: h^T[F, CAP] = gelu(w1^T @ x^T + b1)
            gelu = mybir.ActivationFunctionType.Gelu
            h_sb = []
            for m1 in range(MT1):
                pss = [psA.tile([P, b - a], f32, tag=f"psA{i}",
                                name=f"psA{i}_{m1}")
                       for i, (a, b) in enumerate(CHUNKS_A)]
                for k in range(KT1):
                    lhs = xw_sb[k][:, C_W1 + m1 * P:C_W1 + (m1 + 1) * P]
                    st, sp = (k == 0), (k == KT1 - 1)
                    for i, (a, b) in enumerate(CHUNKS_A):
                        nc.tensor.matmul(pss[i][:], lhs,
                                         xw_sb[k][:, C_X + a:C_X + b],
                                         start=st, stop=sp)
                h = hp.tile([P, CAP], mm_dt, tag="h", name=f"h_{m1}")
                bias = gb_sb[:, m1:m1 + 1]
                for i, (a, b) in enumerate(CHUNKS_A):
                    nc.scalar.activation(h[:, a:b], pss[i][:], gelu, bias=bias)
                h_sb.append(h)

            # Phase B: y[CAP, D] = (h @ w2) * gate
            for m in range(NT):
                pss = [psB.tile([P, 512], f32, tag="psB", name=f"psB_{m}_{n}")
                       for n in range(2)]
                for k2 in range(MT1):
                    lhs = h_sb[k2][:, m * P:(m + 1) * P]
                    w2base = C_W2 + (k2 % 2) * D
                    st, sp = (k2 == 0), (k2 == MT1 - 1)
                    for n in range(2):
                        nc.tensor.matmul(
                            pss[n][:], lhs,
                            xw_sb[k2 // 2][:, w2base + n * 512:
                                           w2base + (n + 1) * 512],
                            start=st, stop=sp)
                gate = gb_sb[:, MT1 + m:MT1 + m + 1]
                for n in range(2):
                    for c in range(2):
                        stage = stp.tile([P, 256], f32, tag="stage",
                                         name=f"stage_{m}_{n}_{c}")
                        nc.vector.tensor_scalar_mul(
                            stage[:], pss[n][:, c * 256:(c + 1) * 256], gate)
                        col = n * 512 + c * 256
                        nc.sync.dma_start(
                            out=y[m * P:(m + 1) * P, col:col + 256],
                            in_=stage[:])
    if not nc.is_finalized():
        nc.finalize()
    return nc


def _get_nc():
    if "nc" not in _NC_CACHE:
        _NC_CACHE["nc"] = _build_bass()
    return _NC_CACHE["nc"]


def kernel(x, dispatch_tensor, combine_tensor, w1, b1, w2, b2, **_):
    from concourse.bass_utils import run_bass_kernel_spmd

    x = np.ascontiguousarray(np.asarray(x, dtype=np.float32)).reshape(T, D)
    dispatch = np.asarray(dispatch_tensor, dtype=np.float32).reshape(T, E)
    combine = np.asarray(combine_tensor, dtype=np.float32).reshape(T, E)
    w1 = np.asarray(w1, dtype=np.float32)
    b1 = np.asarray(b1, dtype=np.float32)
    w2 = np.asarray(w2, dtype=np.float32)
    b2 = np.asarray(b2, dtype=np.float32)

    top = dispatch.argmax(-1)
    gate = combine.sum(-1)
    idxs = [np.nonzero(top == e)[0] for e in range(E)]

    in_maps = []
    for e in range(E):
        idx = idxs[e]
        c = len(idx)
        assert c <= CAP, f"expert {e} got {c} tokens > CAP={CAP}"
        xw = np.zeros((D, XW), np.float32)
        xw[:, C_X:C_X + c] = x[idx].T
        xw[:, C_W1:C_W1 + F] = w1[e]
        for k in range(KT1):
            xw[k * P:(k + 1) * P, C_W2:C_W2 + D] = \
                w2[e][(2 * k) * P:(2 * k + 1) * P, :]
            xw[k * P:(k + 1) * P, C_W2 + D:C_W2 + 2 * D] = \
                w2[e][(2 * k + 1) * P:(2 * k + 2) * P, :]
        if MM_DT == "float32r":
            xw = _round_fp32r(xw)
        gb = np.zeros((P, MT1 + NT), np.float32)
        gb[:, :MT1] = b1[e].reshape(MT1, P).T
        g = np.zeros(CAP, np.float32)
        g[:c] = gate[idx]
        gb[:, MT1:] = g.reshape(NT, P).T
        in_maps.append({"xw": xw, "gb": gb})

    global _LAST_IN_MAPS
    _LAST_IN_MAPS = in_maps
    nc = _get_nc()
    res = run_bass_kernel_spmd(nc, in_maps, list(range(E)))

    y_flat = np.empty((T, D), np.float32)
    for e in range(E):
        y_flat[idxs[e]] = res.results[e]["y"][:len(idxs[e])]
    return (y_flat + b2[None, :]).reshape(B, N, D)


# revision 12
# speedup vs baseline: 3.2386x; 1.4143x over previous
"""Trainium2 Bass kernel for top-1 MoE expert MLP (nn_Experts problem).

Strategy (expert-parallel, one expert per NeuronCore):
  - Routing is one-hot top-1: each token is processed by exactly one expert,
    so each core computes the MLP only for the tokens routed to its expert.
  - Host-side shard step: compute token->expert assignment from
    dispatch_tensor, gather each expert's tokens (transposed to [D, CAP]),
    and pack w1 into per-m1 column blocks so every DMA is contiguous and
    arrives in the order compute consumes it (HWDGE executes FIFO per
    issuing engine, so issue order == arrival order).
  - Device: h^T[F,CAP] = gelu(w1^T @ xT + b1); y[CAP,D] = (h @ w2) * gate.
  - Host-side unshard step: scatter per-expert rows back to token order and
    add the shared output bias b2.  No cross-core reduction is needed since
    token outputs are disjoint across experts.

MM_DT selects the tensor-engine dtype: "float32" is exact (4 cycles/row),
"float32r" streams at full rate (1 cycle/row, ~4x faster) but rounds matmul
operands to a 12-bit significand (~2e-4 relative).  For float32r the host
pre-rounds the operands (round-to-nearest-even at bit 12) so the BIR
verifier sees fp32r-clean inputs.
"""

import numpy as np

B, N, D, E, F = 8, 512, 1024, 8, 2048
T = B * N
P = 128
CAP = 640            # per-expert token capacity (max observed ~549 for T=4096, E=8)
NT = CAP // P        # 5 token tiles per expert
KT1 = D // P         # 8  k-tiles for matmul1 (contract over D)
MT1 = F // P         # 16 m-tiles for matmul1 / k-tiles for matmul2
MM_DT = "float32r"   # matmul dtype: "float32" (exact) or "float32r" (fast)

# phase-A token chunking (fp32r needs moving dim >= 256 for full rate)
CHUNKS_A = ((0, 320), (320, 640))

_NC_CACHE = {}


def _round_fp32r(a):
    """Round-to-nearest-even at mantissa bit 12 (fp32r keeps the top 20 bits
    of an fp32 word: 1 sign + 8 exp + 11 explicit mantissa bits)."""
    u = a.view(np.uint32)
    lsb = (u >> 12) & 1
    u = u + 0x7FF + lsb
    u &= np.uint32(0xFFFFF000)
    return u.view(np.float32)


def _build_bass():
    import concourse.bacc as bacc
    import concourse.tile as tile
    from concourse import mybir

    f32 = mybir.dt.float32
    mm_dt = getattr(mybir.dt, MM_DT)

    nc = bacc.Bacc(None, target_bir_lowering=False)
    gb = nc.declare_dram_parameter("gb", [P, MT1 + NT], f32, isOutput=False)
    xT = nc.declare_dram_parameter("xT", [D, CAP], mm_dt, isOutput=False)
    w1s = nc.declare_dram_parameter("w1s", [F, D], mm_dt, isOutput=False)
    w2s = nc.declare_dram_parameter("w2s", [F, D], mm_dt, isOutput=False)
    y = nc.declare_dram_parameter("y", [CAP, D], f32, isOutput=True)

    with tile.TileContext(nc) as tc:
        with (
            tc.tile_pool(name="gbp", bufs=1) as gbp,
            tc.tile_pool(name="xp", bufs=KT1) as xp,
            tc.tile_pool(name="w1p", bufs=6) as w1p,
            tc.tile_pool(name="w2p", bufs=MT1) as w2p,
            tc.tile_pool(name="hp", bufs=MT1) as hp,
            tc.tile_pool(name="stp", bufs=4) as stp,
            tc.tile_pool(name="psA", bufs=2, space="PSUM") as psA,
            tc.tile_pool(name="psB", bufs=4, space="PSUM") as psB,
        ):
            # DMA issue order == HWDGE arrival order: gb, xT[0], w1[0] (so
            # the first matmul can start ~2us in), the rest of xT, the rest
            # of w1 (phase A stream), then w2 (landing during phase A).
            gb_sb = gbp.tile([P, MT1 + NT], f32)
            nc.sync.dma_start(out=gb_sb[:], in_=gb[:, :])

            def load_x(k):
                t = xp.tile([P, CAP], mm_dt, tag="x", name=f"x_{k}")
                nc.sync.dma_start(out=t[:], in_=xT[k * P:(k + 1) * P, :])
                return t

            def load_w1(m1):
                t = w1p.tile([P, KT1 * P], mm_dt, tag="w1", name=f"w1_{m1}")
                nc.sync.dma_start(out=t[:], in_=w1s[m1 * P:(m1 + 1) * P, :])
                return t

            x_sb = [load_x(0)]
            w1_sb = [load_w1(0)]
            for k in range(1, KT1):
                x_sb.append(load_x(k))
            for m1 in range(1, MT1):
                w1_sb.append(load_w1(m1))
            w2_sb = []
            for k2 in range(MT1):
                t = w2p.tile([P, D], mm_dt, tag="w2", name=f"w2_{k2}")
                nc.sync.dma_start(out=t[:], in_=w2s[k2 * P:(k2 + 1) * P, :])
                w2_sb.append(t)

            # Phase A: h^T[F, CAP] = gelu(w1^T @ x^T + b1)
            gelu = mybir.ActivationFunctionType.Gelu
            h_sb = []
            for m1 in range(MT1):
                pss = [psA.tile([P, b - a], f32, tag=f"psA{i}",
                                name=f"psA{i}_{m1}")
                       for i, (a, b) in enumerate(CHUNKS_A)]
                for k in range(KT1):
                    lhs = w1_sb[m1][:, k * P:(k + 1) * P]
                    st, sp = (k == 0), (k == KT1 - 1)
                    for i, (a, b) in enumerate(CHUNKS_A):
                        nc.tensor.matmul(pss[i][:], lhs, x_sb[k][:, a:b],
                                         start=st, stop=sp)
                h = hp.tile([P, CAP], mm_dt, tag="h", name=f"h_{m1}")
                bias = gb_sb[:, m1:m1 + 1]
                for i, (a, b) in enumerate(CHUNKS_A):
                    nc.scalar.activation(h[:, a:b], pss[i][:], gelu, bias=bias)
                h_sb.append(h)

            # Phase B: y[CAP, D] = (h @ w2) * gate
            for m in range(NT):
                pss = [psB.tile([P, 512], f32, tag="psB", name=f"psB_{m}_{n}")
                       for n in range(2)]
                for k2 in range(MT1):
                    lhs = h_sb[k2][:, m * P:(m + 1) * P]
                    st, sp = (k2 == 0), (k2 == MT1 - 1)
                    for n in range(2):
                        nc.tensor.matmul(pss[n][:], lhs,
                                         w2_sb[k2][:, n * 512:(n + 1) * 512],
                                         start=st, stop=sp)
                gate = gb_sb[:, MT1 + m:MT1 + m + 1]
                for n in range(2):
                    stage = stp.tile([P, 512], f32, tag="stage",
                                     name=f"stage_{m}_{n}")
                    nc.vector.tensor_scalar_mul(stage[:], pss[n][:], gate)
                    nc.sync.dma_start(
                        out=y[m * P:(m + 1) * P, n * 512:(n + 1) * 512],
                        in_=stage[:])
    if not nc.is_finalized():
        nc.finalize()
    return nc


def _get_nc():
    if "nc" not in _NC_CACHE:
        _NC_CACHE["nc"] = _build_bass()
    return _NC_CACHE["nc"]


def kernel(x, dispatch_tensor, combine_tensor, w1, b1, w2, b2, **_):
    from concourse.bass_utils import run_bass_kernel_spmd

    x = np.ascontiguousarray(np.asarray(x, dtype=np.float32)).reshape(T, D)
    dispatch = np.asarray(dispatch_tensor, dtype=np.float32).reshape(T, E)
    combine = np.asarray(combine_tensor, dtype=np.float32).reshape(T, E)
    w1 = np.asarray(w1, dtype=np.float32)
    b1 = np.asarray(b1, dtype=np.float32)
    w2 = np.asarray(w2, dtype=np.float32)
    b2 = np.asarray(b2, dtype=np.float32)

    top = dispatch.argmax(-1)
    gate = combine.sum(-1)
    idxs = [np.nonzero(top == e)[0] for e in range(E)]

    rnd = _round_fp32r if MM_DT == "float32r" else (lambda a: a)
    in_maps = []
    for e in range(E):
        idx = idxs[e]
        c = len(idx)
        assert c <= CAP, f"expert {e} got {c} tokens > CAP={CAP}"
        xT = np.zeros((D, CAP), np.float32)
        xT[:, :c] = x[idx].T
        # w1s[m1*P+p, k*P+m] = w1[k*P+p, m1*P+m]: per-m1 [P, D] blocks whose
        # [:, k*P:(k+1)*P] slice is the lhsT k-tile for output tile m1.
        w1s = np.ascontiguousarray(
            w1[e].reshape(KT1, P, MT1, P).transpose(2, 1, 0, 3)
        ).reshape(F, D)
        gb = np.zeros((P, MT1 + NT), np.float32)
        gb[:, :MT1] = b1[e].reshape(MT1, P).T
        g = np.zeros(CAP, np.float32)
        g[:c] = gate[idx]
        gb[:, MT1:] = g.reshape(NT, P).T
        in_maps.append({
            "gb": gb,
            "xT": rnd(xT),
            "w1s": rnd(w1s),
            "w2s": rnd(np.ascontiguousarray(w2[e])),
        })

    global _LAST_IN_MAPS
    _LAST_IN_MAPS = in_maps
    nc = _get_nc()
    res = run_bass_kernel_spmd(nc, in_maps, list(range(E)))

    y_flat = np.empty((T, D), np.float32)
    for e in range(E):
        y_flat[idxs[e]] = res.results[e]["y"][:len(idxs[e])]
    return (y_flat + b2[None, :]).reshape(B, N, D)
